# revision 66
# baseline (speedup 1.0000x reference)
"""Trainium2 Bass kernel for nn_DHT_Layer (conv1x1+BN+ReLU -> Deep Hough
Transform -> two 3x3 conv+BN+ReLU layers).

Sharding: data-parallel over batch. 8 images / 8 cores -> one image per core,
no collectives; full inputs in, full output out. Inside each core:
  conv1   : 1x1 conv as 2 K-chunk matmuls (bf16), BN+ReLU folded into the
            per-channel scale/bias epilogue on the scalar engine.
  DHT     : out[c,a,r] = sum_p h[c,p] * (idx[a,p]==r) as one-hot matmuls.
            h1 is transposed to [pixel, channel] chunks (PE transposes,
            stationary operand).  The one-hot is *windowed*: along the
            contraction axis the rho index moves <= ~0.5/pixel (the axis --
            image rows or columns -- is chosen per angle), so a 128-pixel
            chunk touches only a ~30-50 wide bin window.  That cuts the
            streamed matmul columns ~8x vs a dense one-hot.  Angles are
            packed in groups of 4 neighbours sharing a window; a group
            accumulates into one PSUM bank with bins interleaved as
            psum_col = r*4+i so every matmul out-AP is a contiguous slice.
            One-hot source is hybrid to balance engine load: ~half streamed
            as a pre-expanded bf16 table over DMA (SP- and Pool-issued),
            half built on the vector engine via is_equal(j_iota, idx_rel)
            with broadcast access patterns.
  conv2/3 : 3x3 convs as 9 shifted matmuls over a zero-padded [c, 102*102]
            layout, BN+ReLU folded into the epilogue.

Cost-model device time: ~236 us/core (PE-bound at ~85% occupancy).

The local walrus build only supports ONE sync-wait per instruction, so a
post-pass splits multi-wait instructions into single-wait NoOp carriers.
"""

import functools
import math

import ml_dtypes
import numpy as np

N = 8          # batch / cores
CIN = 256
CMID = 128
H = W = 100
HW = H * W
A = 100        # angles
R = 100        # rho bins
P = 128
NCHUNK = (HW + P - 1) // P   # 79 pixel chunks of 128
TAIL = HW - (NCHUNK - 1) * P  # 16 valid pixels in last chunk
PADW = W + 2                  # 102 padded spatial for 3x3 convs
BN_EPS = 1e-5
GSIZE = 4      # angles per group (one PSUM bank, 4 slots of 128)
SLOT = 128
BF16 = ml_dtypes.bfloat16


# ----------------------------------------------------------------------------
# host-side precomputation (shapes are fixed -> cache)
# ----------------------------------------------------------------------------

def _hough_idx():
    irho = int(math.sqrt(H * H + W * W) + 1) / float(R)
    theta = np.arange(A) * (math.pi / A)
    tab_cos = np.cos(theta) / irho
    tab_sin = np.sin(theta) / irho
    yy, xx = np.meshgrid(np.arange(H) - H // 2, np.arange(W) - W // 2,
                         indexing='ij')
    xxf = xx.reshape(-1).astype(np.float64)
    yyf = yy.reshape(-1).astype(np.float64)
    r = np.round(xxf[None, :] * tab_cos[:, None] + yyf[None, :] * tab_sin[:, None])
    idx = np.clip((r + R // 2).astype(np.int32), 0, R - 1)  # [A, HW] row-major
    return idx, tab_cos, tab_sin


def _consecutive_runs(vals):
    runs = []
    cur = [vals[0]]
    for v in vals[1:]:
        if v == cur[-1] + 1:
            cur.append(v)
        else:
            runs.append(cur)
            cur = [v]
    runs.append(cur)
    return runs


@functools.lru_cache(maxsize=1)
def _dht_tables():
    idx, tab_cos, tab_sin = _hough_idx()
    # row-major contraction (pixels advance along x) is narrow when |cos| small
    rm_mask = np.abs(tab_cos) <= np.abs(tab_sin)
    idx_cm = idx.reshape(A, H, W).transpose(0, 2, 1).reshape(A, HW)

    groups = []
    for layout in ('rm', 'cm'):
        alist = [a for a in range(A) if (rm_mask[a] if layout == 'rm' else not rm_mask[a])]
        for run in _consecutive_runs(alist):
            for i in range(0, len(run), GSIZE):
                g_angles = run[i:i + GSIZE]
                src = idx if layout == 'rm' else idx_cm
                gidx = src[g_angles]                      # [g, HW]
                gpad = np.zeros((len(g_angles), NCHUNK * P), np.int32)
                gpad[:, :HW] = gidx
                gc = gpad.reshape(len(g_angles), NCHUNK, P)
                # windows over VALID pixels only (tail chunk: first TAIL rows)
                lo = gc.min(axis=(0, 2))
                hi = gc.max(axis=(0, 2))
                lo[-1] = gc[:, -1, :TAIL].min()
                hi[-1] = gc[:, -1, :TAIL].max()
                win = int((hi - lo + 1).max())
                lo = np.minimum(lo, SLOT - win).astype(np.int32)
                groups.append(dict(layout=layout, angles=g_angles, win=win,
                                   lo=lo, a0=g_angles[0]))

    rm_groups = [g for g in groups if g['layout'] == 'rm']
    cm_groups = [g for g in groups if g['layout'] == 'cm']
    passes = [rm_groups[i:i + 4] for i in range(0, len(rm_groups), 4)] + \
             [cm_groups[i:i + 4] for i in range(0, len(cm_groups), 4)]

    # idx_rel table for the on-chip one-hot build: [128, sum_g 79*gl] bf16,
    # group-major, within a group chunk-major then angle: col = base + k*gl + i
    # value = idx[angle, pixel(k, p)] - lo[group, k]  (in [0, win));
    # tail-chunk invalid rows get value -1 (never equal to any j >= 0).
    parts = []
    base = 0
    groups_flat = [g for pss in passes for g in pss]
    for g in groups_flat:
        gl = len(g['angles'])
        src = idx if g['layout'] == 'rm' else idx_cm
        arr = np.full((P, NCHUNK, gl), -1.0, np.float32)
        for ii, a in enumerate(g['angles']):
            vals = np.full(NCHUNK * P, -1.0, np.float32)
            vals[:HW] = src[a].astype(np.float32)
            v = vals.reshape(NCHUNK, P).T           # [P, NCHUNK]
            rel = v - g['lo'][None, :]
            rel[v < 0] = -1.0
            arr[:, :, ii] = rel
        g['base'] = base
        base += NCHUNK * gl
        parts.append(arr.reshape(P, NCHUNK * gl))
    idxrel = np.ascontiguousarray(
        np.concatenate(parts, axis=1).astype(BF16))   # [128, base]
    jiota = np.ascontiguousarray(
        np.tile(np.arange(SLOT, dtype=np.float32), (P, 1)).astype(BF16))

    # hybrid sourcing: ~5/8 of the one-hot volume comes as a pre-expanded
    # bf16 table over DMA (SP is otherwise idle), the rest is built on DVE
    # with is_equal (Pool can't run TensorTensor on this ISA).
    # per-pass source mix so the per-block feed rate is balanced between
    # SP-issued DMA, Pool-issued DMA, and DVE is_equal builds
    for pi, pss in enumerate(passes):
        pat = ['sync', 'dve', 'pool', 'dve' if pi % 2 == 0 else 'sync']
        for gi, g in enumerate(pss):
            g['src'] = pat[gi % 4]
            g['dma'] = g['src'] != 'dve'
    # expanded table for DMA groups, block layout (k0-block of BLDK) matching
    # the on-chip consumption: cols = (k within block, j, i)
    tparts = []
    cursor = 0
    jr = np.arange(SLOT)
    for g in groups_flat:
        if not g['dma']:
            continue
        gl = len(g['angles'])
        win = g['win']
        src = idx if g['layout'] == 'rm' else idx_cm
        g['tbase'] = cursor
        onehot = np.zeros((P, NCHUNK, win, gl), np.float32)
        for ii, a in enumerate(g['angles']):
            vals = np.full(NCHUNK * P, -1.0, np.float32)
            vals[:HW] = src[a].astype(np.float32)
            v = vals.reshape(NCHUNK, P).T               # [P, NCHUNK]
            rel = v - g['lo'][None, :]
            rel[v < 0] = -1.0
            onehot[:, :, :, ii] = rel[:, :, None] == jr[None, None, :win]
        tparts.append(onehot.reshape(P, NCHUNK * win * gl))
        cursor += NCHUNK * win * gl
    ohtable = (np.ascontiguousarray(np.concatenate(tparts, 1).astype(BF16))
               if tparts else np.zeros((P, 1), BF16))
    return dict(passes=passes, idxrel=idxrel, jiota=jiota, ohtable=ohtable)


def _prep_weights(w1, b1, g1, be1, m1, v1, w2, b2, g2, be2, m2, v2,
                  w3, b3, g3, be3, m3, v3):
    s1 = g1 / np.sqrt(v1 + BN_EPS)
    s2 = g2 / np.sqrt(v2 + BN_EPS)
    s3 = g3 / np.sqrt(v3 + BN_EPS)
    # conv1: y[co] = sum_ci w1[co,ci]*x[ci]; fold BN scale into co rows.
    w1f = (w1[:, :, 0, 0] * s1[:, None]).T            # [ci=256, co=128]
    w1p = np.ascontiguousarray(
        w1f.reshape(2, 128, 128).astype(BF16))        # [half, ci128, co]
    bias1 = ((b1 - m1) * s1 + be1).astype(np.float32).reshape(128, 1)
    # conv2/3: [9 taps][ci, co], scaled by s[co]
    w2f = (w2 * s2[:, None, None, None]).transpose(2, 3, 1, 0)  # [ky,kx,ci,co]
    w2p = np.ascontiguousarray(w2f.reshape(9, 128, 128).astype(BF16))
    bias2 = ((b2 - m2) * s2 + be2).astype(np.float32).reshape(128, 1)
    w3f = (w3 * s3[:, None, None, None]).transpose(2, 3, 1, 0)
    w3p = np.ascontiguousarray(w3f.reshape(9, 128, 128).astype(BF16))
    bias3 = ((b3 - m3) * s3 + be3).astype(np.float32).reshape(128, 1)
    ident = np.eye(128, dtype=BF16)
    return w1p, bias1, w2p, bias2, w3p, bias3, ident


# ----------------------------------------------------------------------------
# walrus workaround: split multi-wait instructions (this build supports only
# one sync-wait per instruction)
# ----------------------------------------------------------------------------

def _split_multi_waits(nc, mybir, max_waits=1):
    cnt = 0
    for f in nc.m.functions:
        for bb in f.blocks:
            insts = list(bb.instructions)
            new = []
            changed = False
            for inst in insts:
                si = inst.sync_info
                if si is not None:
                    ow = list(si.on_wait)
                    if len(ow) > max_waits:
                        changed = True
                        head = ow[:-max_waits]
                        for i in range(0, len(head), max_waits):
                            nop = mybir.InstNoOp(name=f'waitsplit_{cnt}',
                                                 ins=[], outs=[])
                            cnt += 1
                            nop.engine = inst.engine
                            nop.sync_info = mybir.SyncInfo(
                                on_wait=head[i:i + max_waits], on_update=[])
                            new.append(nop)
                        si.on_wait = ow[-max_waits:]
                new.append(inst)
            if changed:
                bb.instructions = new
    return cnt


# ----------------------------------------------------------------------------
# bass program
# ----------------------------------------------------------------------------

_PROGRAM_CACHE = {}


def _build_program(split_waits=True, debug_outs=False, repeat=1):
    key = ('nc', split_waits, debug_outs, repeat)
    if key in _PROGRAM_CACHE:
        return _PROGRAM_CACHE[key]
    import concourse.bass as bass
    import concourse.mybir as mybir
    import concourse.tile as tile
    from contextlib import ExitStack

    T = _dht_tables()
    passes = T['passes']
    idxrel_cols = T['idxrel'].shape[1]
    ohtable_cols = T['ohtable'].shape[1]

    f32 = mybir.dt.float32
    bf16 = mybir.dt.bfloat16
    RELU = mybir.ActivationFunctionType.Relu
    COPY = mybir.ActivationFunctionType.Copy

    nc = bass.Bass('TRN2', target_bir_lowering=False, debug=False)
    x_d = nc.dram_tensor('x', [CIN, HW], bf16, kind='ExternalInput')
    w1_d = nc.dram_tensor('w1p', [2, 128, 128], bf16, kind='ExternalInput')
    b1_d = nc.dram_tensor('bias1', [128, 1], f32, kind='ExternalInput')
    w2_d = nc.dram_tensor('w2p', [9, 128, 128], bf16, kind='ExternalInput')
    b2_d = nc.dram_tensor('bias2', [128, 1], f32, kind='ExternalInput')
    w3_d = nc.dram_tensor('w3p', [9, 128, 128], bf16, kind='ExternalInput')
    b3_d = nc.dram_tensor('bias3', [128, 1], f32, kind='ExternalInput')
    id_d = nc.dram_tensor('ident', [128, 128], bf16, kind='ExternalInput')
    ir_d = nc.dram_tensor('idxrel', [128, idxrel_cols], bf16,
                          kind='ExternalInput')
    ji_d = nc.dram_tensor('jiota', [128, SLOT], bf16, kind='ExternalInput')
    tb_d = nc.dram_tensor('ohtable', [128, ohtable_cols], bf16,
                          kind='ExternalInput')
    out_d = nc.dram_tensor('out', [128, HW], f32, kind='ExternalOutput')
    if debug_outs:
        dbg_h1_d = nc.dram_tensor('dbg_h1', [128, HW], bf16,
                                  kind='ExternalOutput')
        dbg_h1t_d = nc.dram_tensor('dbg_h1t', [128, NCHUNK * 128], bf16,
                                   kind='ExternalOutput')
        dbg_dht_d = nc.dram_tensor('dbg_dht', [128, PADW * PADW], bf16,
                                   kind='ExternalOutput')
        dbg_h2_d = nc.dram_tensor('dbg_h2', [128, PADW * PADW], bf16,
                                  kind='ExternalOutput')

    with tile.TileContext(nc) as tc, ExitStack() as st0:
        consts = st0.enter_context(tc.tile_pool(name='consts', bufs=1))
        h1t_pool = st0.enter_context(tc.tile_pool(name='h1t', bufs=1))
        pad_pool = st0.enter_context(tc.tile_pool(name='pads', bufs=1))
        outb_pool = st0.enter_context(tc.tile_pool(name='outb', bufs=3))

        w1_t = consts.tile([128, 2 * 128], bf16, tag='w1')
        nc.sync.dma_start(out=w1_t[:, 0:128], in_=w1_d.ap()[0])
        nc.sync.dma_start(out=w1_t[:, 128:256], in_=w1_d.ap()[1])
        w2_t = consts.tile([128, 9 * 128], bf16, tag='w2')
        w3_t = consts.tile([128, 9 * 128], bf16, tag='w3')
        b1_t = consts.tile([128, 1], f32, tag='b1')
        b2_t = consts.tile([128, 1], f32, tag='b2')
        b3_t = consts.tile([128, 1], f32, tag='b3')
        nc.sync.dma_start(out=b1_t[:], in_=b1_d.ap())
        nc.sync.dma_start(out=b2_t[:], in_=b2_d.ap())
        nc.sync.dma_start(out=b3_t[:], in_=b3_d.ap())
        id_t = consts.tile([128, 128], bf16, tag='ident')
        nc.sync.dma_start(out=id_t[:], in_=id_d.ap())
        ir_t = consts.tile([128, idxrel_cols], bf16, tag='idxrel')
        ji_t = consts.tile([128, SLOT], bf16, tag='jiota')
        zero_t = consts.tile([128, 512], bf16, tag='zeros')
        nc.vector.memset(zero_t[:], 0.0)

        h1T_rm = h1t_pool.tile([128, NCHUNK * 128], bf16, tag='h1T_rm')
        h1T_cm = h1t_pool.tile([128, NCHUNK * 128], bf16, tag='h1T_cm')
        # zero the tail chunk's stale rows (garbage * onehot-zero must be 0,
        # and bf16 garbage could be NaN)
        nc.vector.memset(h1T_rm[:, (NCHUNK - 1) * 128:], 0.0)
        nc.vector.memset(h1T_cm[:, (NCHUNK - 1) * 128:], 0.0)

        dht_pad = pad_pool.tile([128, PADW * PADW], bf16, tag='dht_pad')
        h2_pad = pad_pool.tile([128, PADW * PADW], bf16, tag='h2_pad')
        # zero only the borders; the interior is fully overwritten
        for pad_t in (dht_pad, h2_pad):
            pv = pad_t[:].rearrange('c (a r) -> c a r', a=PADW)
            nc.gpsimd.memset(pv[:, 0:1, :], 0.0)
            nc.gpsimd.memset(pv[:, PADW - 1:PADW, :], 0.0)
            nc.gpsimd.memset(pv[:, :, 0:1], 0.0)
            nc.gpsimd.memset(pv[:, :, PADW - 1:PADW], 0.0)

        # ------------------------------------------------ pipeline body
        first_rep = [True]
        ir_pending = [True]

        def emit_pipeline():
          with ExitStack() as stT:
            pst = stT.enter_context(
                tc.tile_pool(name='pst', bufs=2, space='PSUM'))
            h1_pool = stT.enter_context(tc.tile_pool(name='h1', bufs=1))
            h1 = h1_pool.tile([128, HW], bf16, tag='h1')
            h1cm = h1_pool.tile([128, HW], bf16, tag='h1cm')

            with ExitStack() as st1:
                xf_pool = st1.enter_context(tc.tile_pool(name='xf', bufs=6))
                ps1 = st1.enter_context(
                    tc.tile_pool(name='ps1', bufs=2, space='PSUM'))
                sizes = [500] * 20
                cs0 = 0
                for c, CS in enumerate(sizes):
                    sl = slice(cs0, cs0 + CS)
                    cs0 += CS
                    ps = ps1.tile([128, 500], f32, tag='ps1')
                    for hh in range(2):
                        xf = xf_pool.tile([128, 500], bf16, tag='xf')
                        dma_eng = (nc.sync, nc.gpsimd,
                                   nc.scalar)[(2 * c + hh) % 3]
                        dma_eng.dma_start(
                            out=xf[:, :CS],
                            in_=x_d.ap()[hh * 128:(hh + 1) * 128, sl])
                        nc.tensor.matmul(
                            out=ps[:, :CS],
                            lhsT=w1_t[:, hh * 128:(hh + 1) * 128],
                            rhs=xf[:, :CS], start=(hh == 0), stop=(hh == 1))
                    nc.scalar.activation(out=h1[:, sl], in_=ps[:, :CS],
                                         func=RELU, bias=b1_t[:, :1],
                                         scale=1.0)
                    if c == 0 and ir_pending[0]:
                        ir_pending[0] = False
                        nc.sync.dma_start(out=ji_t[:], in_=ji_d.ap())
                        nc.sync.dma_start(out=ir_t[:], in_=ir_d.ap())

            if debug_outs:
                nc.sync.dma_start(out=dbg_h1_d.ap(), in_=h1[:])

            def transposes(src, dst):
                # 4 transposed chunks per PSUM tile -> one ACT copy per 4
                for k0 in range(0, NCHUNK, 4):
                    kc = min(4, NCHUNK - k0)
                    pt = pst.tile([128, 512], bf16, tag='pt', space='PSUM')
                    for kk in range(kc):
                        k = k0 + kk
                        npx = TAIL if k == NCHUNK - 1 else 128
                        nc.tensor.transpose(
                            out=pt[:npx, kk * 128:(kk + 1) * 128],
                            in_=src[:, k * 128:k * 128 + npx],
                            identity=id_t[:])
                    if k0 + kc == NCHUNK:
                        # tail chunk: only TAIL partitions are valid; the
                        # memset zeros in dst rows TAIL.. must survive
                        if kc > 1:
                            nc.scalar.copy(
                                out=dst[:, k0 * 128:(k0 + kc - 1) * 128],
                                in_=pt[:, :(kc - 1) * 128])
                        nc.scalar.copy(
                            out=dst[:TAIL, (NCHUNK - 1) * 128:NCHUNK * 128],
                            in_=pt[:TAIL, (kc - 1) * 128:kc * 128])
                    else:
                        nc.scalar.copy(
                            out=dst[:, k0 * 128:(k0 + kc) * 128],
                            in_=pt[:, :kc * 128])

            transposes(h1, h1T_rm)

            # ------------------------------------------ DHT
            with ExitStack() as st2:
                oh_pool = st2.enter_context(tc.tile_pool(name='oh', bufs=10))
                psd = st2.enter_context(
                    tc.tile_pool(name='psd', bufs=6, space='PSUM'))

                BLD = 10  # chunks per one-hot build/DMA block

                def emit_pass(pss):
                    h1T = h1T_rm if pss[0]['layout'] == 'rm' else h1T_cm
                    ptiles = []
                    for gi, g in enumerate(pss):
                        pt = psd.tile([128, 512], f32, tag='psd',
                                      space='PSUM')
                        # zero + set has_written via K=1 zero matmul
                        nc.tensor.matmul(out=pt[:], lhsT=zero_t[:1, :128],
                                         rhs=zero_t[:1, :512], start=True,
                                         stop=False, skip_group_check=True)
                        ptiles.append(pt)
                    for k0 in range(0, NCHUNK, BLD):
                        cnt = min(BLD, NCHUNK - k0)
                        ohs = []
                        for gi, g in enumerate(pss):
                            gl = len(g['angles'])
                            win = g['win']
                            oh = oh_pool.tile([128, BLD * 52 * GSIZE], bf16,
                                              tag='oh')
                            if g['dma']:
                                c0 = g['tbase'] + k0 * win * gl
                                deng = (nc.sync if g['src'] == 'sync'
                                        else nc.gpsimd)
                                deng.dma_start(
                                    out=oh[:, :cnt * win * gl],
                                    in_=tb_d.ap()[:, c0:c0 + cnt * win * gl])
                            else:
                                # one-hot [128,cnt,win,gl] = (j == idxrel)
                                ov = oh[:, :cnt * win * gl].rearrange(
                                    'p (k j i) -> p k j i', k=cnt, j=win)
                                src_ir = ir_t[:, g['base'] + k0 * gl:
                                              g['base'] + (k0 + cnt) * gl]
                                irv = src_ir.rearrange(
                                    'p (k i) -> p k i', i=gl).unsqueeze(
                                    2).to_broadcast([128, cnt, win, gl])
                                jiv = ji_t[:, :win].unsqueeze(1).unsqueeze(
                                    3).to_broadcast([128, cnt, win, gl])
                                nc.vector.tensor_tensor(
                                    out=ov, in0=jiv, in1=irv,
                                    op=mybir.AluOpType.is_equal)
                            ohs.append(oh)
                        # chunk-major so consecutive matmuls share lhsT
                        # (real HW reloads stationary weights per matmul)
                        for kk in range(cnt):
                            k = k0 + kk
                            for gi, g in enumerate(pss):
                                gl = len(g['angles'])
                                win = g['win']
                                lo = int(g['lo'][k])
                                nc.tensor.matmul(
                                    out=ptiles[gi][:,
                                                   gl * lo:gl * (lo + win)],
                                    lhsT=h1T[:, k * 128:(k + 1) * 128],
                                    rhs=ohs[gi][:, kk * win * gl:
                                                (kk + 1) * win * gl],
                                    start=False, stop=False,
                                    skip_group_check=True)
                    # copy accumulators into conv2 input (de-interleave)
                    for gi, g in enumerate(pss):
                        gl = len(g['angles'])
                        a0 = g['a0']
                        pv = ptiles[gi][:, :gl * 128].rearrange(
                            'p (r s) -> p s r', s=gl)
                        dv = dht_pad[:].rearrange('c (a r) -> c a r', a=PADW)
                        nc.scalar.activation(
                            out=dv[:, a0 + 1:a0 + 1 + gl, 1:1 + R],
                            in_=pv[:, :, :R], func=COPY)

                rm_passes = [p for p in passes if p[0]['layout'] == 'rm']
                cm_passes = [p for p in passes if p[0]['layout'] == 'cm']
                for pss in rm_passes:
                    emit_pass(pss)
                # cm prep runs under the rm passes: the strided h1->h1cm copy
                # on ACT, then the cm transposes follow the rm matmul stream
                nc.scalar.activation(
                    out=h1cm[:],
                    in_=h1[:].rearrange('c (y x) -> c x y', y=H, x=W),
                    func=COPY)
                transposes(h1cm, h1T_cm)
                for pss in cm_passes:
                    emit_pass(pss)

          if debug_outs:
            nc.sync.dma_start(out=dbg_h1t_d.ap(), in_=h1T_rm[:])
            nc.sync.dma_start(out=dbg_dht_d.ap(), in_=dht_pad[:])

          # ---------------------------------------------- conv2 / conv3
          with ExitStack() as st3:
            ps2 = st3.enter_context(
                tc.tile_pool(name='ps2', bufs=4, space='PSUM'))
            if first_rep[0]:
                first_rep[0] = False
                for t9 in range(9):
                    nc.sync.dma_start(out=w2_t[:, t9 * 128:(t9 + 1) * 128],
                                      in_=w2_d.ap()[t9])
                    nc.sync.dma_start(out=w3_t[:, t9 * 128:(t9 + 1) * 128],
                                      in_=w3_d.ap()[t9])
            AR = 4  # angle rows per psum chunk
            for conv_i, (w_t, b_t, src_t) in enumerate(
                    ((w2_t, b2_t, dht_pad), (w3_t, b3_t, h2_pad))):
                sv = src_t[:].rearrange('c (a r) -> c a r', a=PADW)
                for c in range(A // AR):
                    a0 = c * AR
                    ps = ps2.tile([128, AR * R], f32, tag='ps2')
                    for t9 in range(9):
                        dy, dx = divmod(t9, 3)
                        nc.tensor.matmul(
                            out=ps[:],
                            lhsT=w_t[:, t9 * 128:(t9 + 1) * 128],
                            rhs=sv[:, a0 + dy:a0 + dy + AR, dx:dx + R],
                            start=(t9 == 0), stop=(t9 == 8))
                    pv = ps[:].rearrange('p (a r) -> p a r', a=AR)
                    if conv_i == 0:
                        hv = h2_pad[:].rearrange('c (a r) -> c a r', a=PADW)
                        nc.scalar.activation(
                            out=hv[:, a0 + 1:a0 + 1 + AR, 1:1 + R],
                            in_=pv[:], func=RELU, bias=b_t[:, :1], scale=1.0)
                        if debug_outs and c == A // AR - 1:
                            nc.sync.dma_start(out=dbg_h2_d.ap(),
                                              in_=h2_pad[:])
                    else:
                        ob = outb_pool.tile([128, AR * R], f32, tag='outb')
                        ov = ob[:].rearrange('p (a r) -> p a r', a=AR)
                        nc.scalar.activation(out=ov[:], in_=pv[:], func=RELU,
                                             bias=b_t[:, :1], scale=1.0)
                        nc.sync.dma_start(
                            out=out_d.ap()[:, a0 * R:(a0 + AR) * R],
                            in_=ob[:])

        for _rep in range(repeat):
            emit_pipeline()

    if split_waits:
        _split_multi_waits(nc, mybir)
    _PROGRAM_CACHE[key] = nc
    return nc


# ----------------------------------------------------------------------------
# entry point
# ----------------------------------------------------------------------------

def make_in_maps(inputs):
    T = _dht_tables()
    x = np.asarray(inputs['x'], np.float32)
    w1p, bias1, w2p, bias2, w3p, bias3, ident = _prep_weights(
        *[np.asarray(inputs[k], np.float32) for k in
          ('w1', 'b1', 'g1', 'be1', 'm1', 'v1',
           'w2', 'b2', 'g2', 'be2', 'm2', 'v2',
           'w3', 'b3', 'g3', 'be3', 'm3', 'v3')])
    common = dict(w1p=w1p, bias1=bias1, w2p=w2p, bias2=bias2, w3p=w3p,
                  bias3=bias3, ident=ident, idxrel=T['idxrel'],
                  jiota=T['jiota'], ohtable=T['ohtable'])
    return [
        {'x': np.ascontiguousarray(x[n]).reshape(CIN, HW).astype(BF16),
         **common}
        for n in range(N)
    ]


def run(inputs, trace=False):
    from concourse.bass_utils import run_bass_kernel_spmd

    nc = _build_program()
    in_maps = make_in_maps(inputs)
    res = run_bass_kernel_spmd(nc, in_maps, core_ids=list(range(N)),
                               trace=trace)
    out = np.stack([res.results[n]['out'].reshape(CMID, H, W)
                    for n in range(N)], axis=0)
    return out.astype(np.float32), res


def kernel(**inputs):
    out, _ = run(inputs, trace=False)
    return out


# revision 68
# speedup vs baseline: 1.0027x; 1.0027x over previous
"""Trainium2 Bass kernel for nn_DHT_Layer (conv1x1+BN+ReLU -> Deep Hough
Transform -> two 3x3 conv+BN+ReLU layers).

Sharding: data-parallel over batch. 8 images / 8 cores -> one image per core,
no collectives; full inputs in, full output out. Inside each core:
  conv1   : 1x1 conv as 2 K-chunk matmuls (bf16), BN+ReLU folded into the
            per-channel scale/bias epilogue on the scalar engine.
  DHT     : out[c,a,r] = sum_p h[c,p] * (idx[a,p]==r) as one-hot matmuls.
            h1 is transposed to [pixel, channel] chunks (PE transposes,
            stationary operand).  The one-hot is *windowed*: along the
            contraction axis the rho index moves <= ~0.5/pixel (the axis --
            image rows or columns -- is chosen per angle), so a 128-pixel
            chunk touches only a ~30-50 wide bin window.  That cuts the
            streamed matmul columns ~8x vs a dense one-hot.  Angles are
            packed in groups of 4 neighbours sharing a window; a group
            accumulates into one PSUM bank with bins interleaved as
            psum_col = r*4+i so every matmul out-AP is a contiguous slice.
            One-hot source is hybrid to balance engine load: ~half streamed
            as a pre-expanded bf16 table over DMA (SP- and Pool-issued),
            half built on the vector engine via is_equal(j_iota, idx_rel)
            with broadcast access patterns.
  conv2/3 : 3x3 convs as 9 shifted matmuls over a zero-padded [c, 102*102]
            layout, BN+ReLU folded into the epilogue.

Cost-model device time: ~236 us/core (PE-bound at ~85% occupancy).

The local walrus build only supports ONE sync-wait per instruction, so a
post-pass splits multi-wait instructions into single-wait NoOp carriers.
"""

import functools
import math

import ml_dtypes
import numpy as np

N = 8          # batch / cores
CIN = 256
CMID = 128
H = W = 100
HW = H * W
A = 100        # angles
R = 100        # rho bins
P = 128
NCHUNK = (HW + P - 1) // P   # 79 pixel chunks of 128
TAIL = HW - (NCHUNK - 1) * P  # 16 valid pixels in last chunk
PADW = W + 2                  # 102 padded spatial for 3x3 convs
BN_EPS = 1e-5
GSIZE = 4      # angles per group (one PSUM bank, 4 slots of 128)
SLOT = 128
BF16 = ml_dtypes.bfloat16


# ----------------------------------------------------------------------------
# host-side precomputation (shapes are fixed -> cache)
# ----------------------------------------------------------------------------

def _hough_idx():
    irho = int(math.sqrt(H * H + W * W) + 1) / float(R)
    theta = np.arange(A) * (math.pi / A)
    tab_cos = np.cos(theta) / irho
    tab_sin = np.sin(theta) / irho
    yy, xx = np.meshgrid(np.arange(H) - H // 2, np.arange(W) - W // 2,
                         indexing='ij')
    xxf = xx.reshape(-1).astype(np.float64)
    yyf = yy.reshape(-1).astype(np.float64)
    r = np.round(xxf[None, :] * tab_cos[:, None] + yyf[None, :] * tab_sin[:, None])
    idx = np.clip((r + R // 2).astype(np.int32), 0, R - 1)  # [A, HW] row-major
    return idx, tab_cos, tab_sin


def _consecutive_runs(vals):
    runs = []
    cur = [vals[0]]
    for v in vals[1:]:
        if v == cur[-1] + 1:
            cur.append(v)
        else:
            runs.append(cur)
            cur = [v]
    runs.append(cur)
    return runs


@functools.lru_cache(maxsize=1)
def _dht_tables():
    idx, tab_cos, tab_sin = _hough_idx()
    # row-major contraction (pixels advance along x) is narrow when |cos| small
    rm_mask = np.abs(tab_cos) <= np.abs(tab_sin)
    idx_cm = idx.reshape(A, H, W).transpose(0, 2, 1).reshape(A, HW)

    groups = []
    for layout in ('rm', 'cm'):
        alist = [a for a in range(A) if (rm_mask[a] if layout == 'rm' else not rm_mask[a])]
        for run in _consecutive_runs(alist):
            for i in range(0, len(run), GSIZE):
                g_angles = run[i:i + GSIZE]
                src = idx if layout == 'rm' else idx_cm
                gidx = src[g_angles]                      # [g, HW]
                gpad = np.zeros((len(g_angles), NCHUNK * P), np.int32)
                gpad[:, :HW] = gidx
                gc = gpad.reshape(len(g_angles), NCHUNK, P)
                # windows over VALID pixels only (tail chunk: first TAIL rows)
                lo = gc.min(axis=(0, 2))
                hi = gc.max(axis=(0, 2))
                lo[-1] = gc[:, -1, :TAIL].min()
                hi[-1] = gc[:, -1, :TAIL].max()
                win = int((hi - lo + 1).max())
                lo = np.minimum(lo, SLOT - win).astype(np.int32)
                groups.append(dict(layout=layout, angles=g_angles, win=win,
                                   lo=lo, a0=g_angles[0]))

    rm_groups = [g for g in groups if g['layout'] == 'rm']
    cm_groups = [g for g in groups if g['layout'] == 'cm']
    passes = [rm_groups[i:i + 4] for i in range(0, len(rm_groups), 4)] + \
             [cm_groups[i:i + 4] for i in range(0, len(cm_groups), 4)]

    # idx_rel table for the on-chip one-hot build: [128, sum_g 79*gl] bf16,
    # group-major, within a group chunk-major then angle: col = base + k*gl + i
    # value = idx[angle, pixel(k, p)] - lo[group, k]  (in [0, win));
    # tail-chunk invalid rows get value -1 (never equal to any j >= 0).
    parts = []
    base = 0
    groups_flat = [g for pss in passes for g in pss]
    for g in groups_flat:
        gl = len(g['angles'])
        src = idx if g['layout'] == 'rm' else idx_cm
        arr = np.full((P, NCHUNK, gl), -1.0, np.float32)
        for ii, a in enumerate(g['angles']):
            vals = np.full(NCHUNK * P, -1.0, np.float32)
            vals[:HW] = src[a].astype(np.float32)
            v = vals.reshape(NCHUNK, P).T           # [P, NCHUNK]
            rel = v - g['lo'][None, :]
            rel[v < 0] = -1.0
            arr[:, :, ii] = rel
        g['base'] = base
        base += NCHUNK * gl
        parts.append(arr.reshape(P, NCHUNK * gl))
    idxrel = np.ascontiguousarray(
        np.concatenate(parts, axis=1).astype(BF16))   # [128, base]
    jiota = np.ascontiguousarray(
        np.tile(np.arange(SLOT, dtype=np.float32), (P, 1)).astype(BF16))

    # hybrid sourcing: ~5/8 of the one-hot volume comes as a pre-expanded
    # bf16 table over DMA (SP is otherwise idle), the rest is built on DVE
    # with is_equal (Pool can't run TensorTensor on this ISA).
    # per-pass source mix so the per-block feed rate is balanced between
    # SP-issued DMA, Pool-issued DMA, and DVE is_equal builds
    for pi, pss in enumerate(passes):
        pat = ['sync', 'dve', 'pool', 'dve' if pi % 2 == 0 else 'sync']
        for gi, g in enumerate(pss):
            g['src'] = pat[gi % 4]
            g['dma'] = g['src'] != 'dve'
    # expanded table for DMA groups, block layout (k0-block of BLDK) matching
    # the on-chip consumption: cols = (k within block, j, i)
    tparts = []
    cursor = 0
    jr = np.arange(SLOT)
    for g in groups_flat:
        if not g['dma']:
            continue
        gl = len(g['angles'])
        win = g['win']
        src = idx if g['layout'] == 'rm' else idx_cm
        g['tbase'] = cursor
        onehot = np.zeros((P, NCHUNK, win, gl), np.float32)
        for ii, a in enumerate(g['angles']):
            vals = np.full(NCHUNK * P, -1.0, np.float32)
            vals[:HW] = src[a].astype(np.float32)
            v = vals.reshape(NCHUNK, P).T               # [P, NCHUNK]
            rel = v - g['lo'][None, :]
            rel[v < 0] = -1.0
            onehot[:, :, :, ii] = rel[:, :, None] == jr[None, None, :win]
        tparts.append(onehot.reshape(P, NCHUNK * win * gl))
        cursor += NCHUNK * win * gl
    ohtable = (np.ascontiguousarray(np.concatenate(tparts, 1).astype(BF16))
               if tparts else np.zeros((P, 1), BF16))
    return dict(passes=passes, idxrel=idxrel, jiota=jiota, ohtable=ohtable)


def _prep_weights(w1, b1, g1, be1, m1, v1, w2, b2, g2, be2, m2, v2,
                  w3, b3, g3, be3, m3, v3):
    s1 = g1 / np.sqrt(v1 + BN_EPS)
    s2 = g2 / np.sqrt(v2 + BN_EPS)
    s3 = g3 / np.sqrt(v3 + BN_EPS)
    # conv1: y[co] = sum_ci w1[co,ci]*x[ci]; fold BN scale into co rows.
    w1f = (w1[:, :, 0, 0] * s1[:, None]).T            # [ci=256, co=128]
    w1p = np.ascontiguousarray(
        w1f.reshape(2, 128, 128).astype(BF16))        # [half, ci128, co]
    bias1 = ((b1 - m1) * s1 + be1).astype(np.float32).reshape(128, 1)
    # conv2/3: [9 taps][ci, co], scaled by s[co]
    w2f = (w2 * s2[:, None, None, None]).transpose(2, 3, 1, 0)  # [ky,kx,ci,co]
    w2p = np.ascontiguousarray(w2f.reshape(9, 128, 128).astype(BF16))
    bias2 = ((b2 - m2) * s2 + be2).astype(np.float32).reshape(128, 1)
    w3f = (w3 * s3[:, None, None, None]).transpose(2, 3, 1, 0)
    w3p = np.ascontiguousarray(w3f.reshape(9, 128, 128).astype(BF16))
    bias3 = ((b3 - m3) * s3 + be3).astype(np.float32).reshape(128, 1)
    ident = np.eye(128, dtype=BF16)
    return w1p, bias1, w2p, bias2, w3p, bias3, ident


# ----------------------------------------------------------------------------
# walrus workaround: split multi-wait instructions (this build supports only
# one sync-wait per instruction)
# ----------------------------------------------------------------------------

def _split_multi_waits(nc, mybir, max_waits=1):
    cnt = 0
    for f in nc.m.functions:
        for bb in f.blocks:
            insts = list(bb.instructions)
            new = []
            changed = False
            for inst in insts:
                si = inst.sync_info
                if si is not None:
                    ow = list(si.on_wait)
                    if len(ow) > max_waits:
                        changed = True
                        head = ow[:-max_waits]
                        for i in range(0, len(head), max_waits):
                            nop = mybir.InstNoOp(name=f'waitsplit_{cnt}',
                                                 ins=[], outs=[])
                            cnt += 1
                            nop.engine = inst.engine
                            nop.sync_info = mybir.SyncInfo(
                                on_wait=head[i:i + max_waits], on_update=[])
                            new.append(nop)
                        si.on_wait = ow[-max_waits:]
                new.append(inst)
            if changed:
                bb.instructions = new
    return cnt


# ----------------------------------------------------------------------------
# bass program
# ----------------------------------------------------------------------------

_PROGRAM_CACHE = {}


def _build_program(split_waits=True, debug_outs=False, repeat=1):
    key = ('nc', split_waits, debug_outs, repeat)
    if key in _PROGRAM_CACHE:
        return _PROGRAM_CACHE[key]
    import concourse.bass as bass
    import concourse.mybir as mybir
    import concourse.tile as tile
    from contextlib import ExitStack

    T = _dht_tables()
    passes = T['passes']
    idxrel_cols = T['idxrel'].shape[1]
    ohtable_cols = T['ohtable'].shape[1]

    f32 = mybir.dt.float32
    bf16 = mybir.dt.bfloat16
    RELU = mybir.ActivationFunctionType.Relu
    COPY = mybir.ActivationFunctionType.Copy

    nc = bass.Bass('TRN2', target_bir_lowering=False, debug=False)
    x_d = nc.dram_tensor('x', [CIN, HW], bf16, kind='ExternalInput')
    w1_d = nc.dram_tensor('w1p', [2, 128, 128], bf16, kind='ExternalInput')
    b1_d = nc.dram_tensor('bias1', [128, 1], f32, kind='ExternalInput')
    w2_d = nc.dram_tensor('w2p', [9, 128, 128], bf16, kind='ExternalInput')
    b2_d = nc.dram_tensor('bias2', [128, 1], f32, kind='ExternalInput')
    w3_d = nc.dram_tensor('w3p', [9, 128, 128], bf16, kind='ExternalInput')
    b3_d = nc.dram_tensor('bias3', [128, 1], f32, kind='ExternalInput')
    id_d = nc.dram_tensor('ident', [128, 128], bf16, kind='ExternalInput')
    ir_d = nc.dram_tensor('idxrel', [128, idxrel_cols], bf16,
                          kind='ExternalInput')
    ji_d = nc.dram_tensor('jiota', [128, SLOT], bf16, kind='ExternalInput')
    tb_d = nc.dram_tensor('ohtable', [128, ohtable_cols], bf16,
                          kind='ExternalInput')
    out_d = nc.dram_tensor('out', [128, HW], f32, kind='ExternalOutput')
    if debug_outs:
        dbg_h1_d = nc.dram_tensor('dbg_h1', [128, HW], bf16,
                                  kind='ExternalOutput')
        dbg_h1t_d = nc.dram_tensor('dbg_h1t', [128, NCHUNK * 128], bf16,
                                   kind='ExternalOutput')
        dbg_dht_d = nc.dram_tensor('dbg_dht', [128, PADW * PADW], bf16,
                                   kind='ExternalOutput')
        dbg_h2_d = nc.dram_tensor('dbg_h2', [128, PADW * PADW], bf16,
                                  kind='ExternalOutput')

    with tile.TileContext(nc) as tc, ExitStack() as st0:
        consts = st0.enter_context(tc.tile_pool(name='consts', bufs=1))
        h1t_pool = st0.enter_context(tc.tile_pool(name='h1t', bufs=1))
        pad_pool = st0.enter_context(tc.tile_pool(name='pads', bufs=1))
        outb_pool = st0.enter_context(tc.tile_pool(name='outb', bufs=3))

        w1_t = consts.tile([128, 2 * 128], bf16, tag='w1')
        nc.sync.dma_start(out=w1_t[:, 0:128], in_=w1_d.ap()[0])
        nc.sync.dma_start(out=w1_t[:, 128:256], in_=w1_d.ap()[1])
        w2_t = consts.tile([128, 9 * 128], bf16, tag='w2')
        w3_t = consts.tile([128, 9 * 128], bf16, tag='w3')
        b1_t = consts.tile([128, 1], f32, tag='b1')
        b2_t = consts.tile([128, 1], f32, tag='b2')
        b3_t = consts.tile([128, 1], f32, tag='b3')
        nc.sync.dma_start(out=b1_t[:], in_=b1_d.ap())
        nc.sync.dma_start(out=b2_t[:], in_=b2_d.ap())
        nc.sync.dma_start(out=b3_t[:], in_=b3_d.ap())
        id_t = consts.tile([128, 128], bf16, tag='ident')
        nc.sync.dma_start(out=id_t[:], in_=id_d.ap())
        ir_t = consts.tile([128, idxrel_cols], bf16, tag='idxrel')
        ji_t = consts.tile([128, SLOT], bf16, tag='jiota')
        zero_t = consts.tile([128, 512], bf16, tag='zeros')
        nc.vector.memset(zero_t[:], 0.0)

        h1T_rm = h1t_pool.tile([128, NCHUNK * 128], bf16, tag='h1T_rm')
        h1T_cm = h1t_pool.tile([128, NCHUNK * 128], bf16, tag='h1T_cm')
        # zero the tail chunk's stale rows (garbage * onehot-zero must be 0,
        # and bf16 garbage could be NaN)
        nc.vector.memset(h1T_rm[:, (NCHUNK - 1) * 128:], 0.0)
        nc.vector.memset(h1T_cm[:, (NCHUNK - 1) * 128:], 0.0)

        dht_pad = pad_pool.tile([128, PADW * PADW], bf16, tag='dht_pad')
        h2_pad = pad_pool.tile([128, PADW * PADW], bf16, tag='h2_pad')
        # zero only the borders; the interior is fully overwritten
        for pad_t in (dht_pad, h2_pad):
            pv = pad_t[:].rearrange('c (a r) -> c a r', a=PADW)
            nc.gpsimd.memset(pv[:, 0:1, :], 0.0)
            nc.gpsimd.memset(pv[:, PADW - 1:PADW, :], 0.0)
            nc.gpsimd.memset(pv[:, :, 0:1], 0.0)
            nc.gpsimd.memset(pv[:, :, PADW - 1:PADW], 0.0)

        # ------------------------------------------------ pipeline body
        first_rep = [True]
        ir_pending = [True]

        def emit_pipeline():
          with ExitStack() as stT:
            pst = stT.enter_context(
                tc.tile_pool(name='pst', bufs=2, space='PSUM'))
            h1_pool = stT.enter_context(tc.tile_pool(name='h1', bufs=1))
            h1 = h1_pool.tile([128, HW], bf16, tag='h1')
            h1cm = h1_pool.tile([128, HW], bf16, tag='h1cm')

            with ExitStack() as st1:
                xf_pool = st1.enter_context(tc.tile_pool(name='xf', bufs=6))
                ps1 = st1.enter_context(
                    tc.tile_pool(name='ps1', bufs=2, space='PSUM'))
                sizes = [500] * 20
                cs0 = 0
                for c, CS in enumerate(sizes):
                    sl = slice(cs0, cs0 + CS)
                    cs0 += CS
                    ps = ps1.tile([128, 500], f32, tag='ps1')
                    for hh in range(2):
                        xf = xf_pool.tile([128, 500], bf16, tag='xf')
                        dma_eng = (nc.sync, nc.gpsimd,
                                   nc.scalar)[(2 * c + hh) % 3]
                        dma_eng.dma_start(
                            out=xf[:, :CS],
                            in_=x_d.ap()[hh * 128:(hh + 1) * 128, sl])
                        nc.tensor.matmul(
                            out=ps[:, :CS],
                            lhsT=w1_t[:, hh * 128:(hh + 1) * 128],
                            rhs=xf[:, :CS], start=(hh == 0), stop=(hh == 1))
                    nc.scalar.activation(out=h1[:, sl], in_=ps[:, :CS],
                                         func=RELU, bias=b1_t[:, :1],
                                         scale=1.0)
                    if c == 0 and ir_pending[0]:
                        ir_pending[0] = False
                        nc.sync.dma_start(out=ji_t[:], in_=ji_d.ap())
                        nc.sync.dma_start(out=ir_t[:], in_=ir_d.ap())

            if debug_outs:
                nc.sync.dma_start(out=dbg_h1_d.ap(), in_=h1[:])

            def transposes(src, dst):
                # 4 transposed chunks per PSUM tile -> one ACT copy per 4
                for k0 in range(0, NCHUNK, 4):
                    kc = min(4, NCHUNK - k0)
                    pt = pst.tile([128, 512], bf16, tag='pt', space='PSUM')
                    for kk in range(kc):
                        k = k0 + kk
                        npx = TAIL if k == NCHUNK - 1 else 128
                        nc.tensor.transpose(
                            out=pt[:npx, kk * 128:(kk + 1) * 128],
                            in_=src[:, k * 128:k * 128 + npx],
                            identity=id_t[:])
                    if k0 + kc == NCHUNK:
                        # tail chunk: only TAIL partitions are valid; the
                        # memset zeros in dst rows TAIL.. must survive
                        if kc > 1:
                            nc.scalar.copy(
                                out=dst[:, k0 * 128:(k0 + kc - 1) * 128],
                                in_=pt[:, :(kc - 1) * 128])
                        nc.scalar.copy(
                            out=dst[:TAIL, (NCHUNK - 1) * 128:NCHUNK * 128],
                            in_=pt[:TAIL, (kc - 1) * 128:kc * 128])
                    else:
                        nc.scalar.copy(
                            out=dst[:, k0 * 128:(k0 + kc) * 128],
                            in_=pt[:, :kc * 128])

            transposes(h1, h1T_rm)

            # ------------------------------------------ DHT
            with ExitStack() as st2:
                oh_pool = st2.enter_context(tc.tile_pool(name='oh', bufs=10))
                psd = st2.enter_context(
                    tc.tile_pool(name='psd', bufs=6, space='PSUM'))

                BLD = 10  # chunks per one-hot build/DMA block

                def emit_pass(pss):
                    h1T = h1T_rm if pss[0]['layout'] == 'rm' else h1T_cm
                    ptiles = []
                    for gi, g in enumerate(pss):
                        pt = psd.tile([128, 512], f32, tag='psd',
                                      space='PSUM')
                        # zero + set has_written via K=1 zero matmul
                        nc.tensor.matmul(out=pt[:], lhsT=zero_t[:1, :128],
                                         rhs=zero_t[:1, :512], start=True,
                                         stop=False, skip_group_check=True)
                        ptiles.append(pt)
                    for k0 in range(0, NCHUNK, BLD):
                        cnt = min(BLD, NCHUNK - k0)
                        ohs = []
                        for gi, g in enumerate(pss):
                            gl = len(g['angles'])
                            win = g['win']
                            oh = oh_pool.tile([128, BLD * 52 * GSIZE], bf16,
                                              tag='oh')
                            if g['dma']:
                                c0 = g['tbase'] + k0 * win * gl
                                deng = (nc.sync if g['src'] == 'sync'
                                        else nc.gpsimd)
                                deng.dma_start(
                                    out=oh[:, :cnt * win * gl],
                                    in_=tb_d.ap()[:, c0:c0 + cnt * win * gl])
                            else:
                                # one-hot [128,cnt,win,gl] = (j == idxrel)
                                ov = oh[:, :cnt * win * gl].rearrange(
                                    'p (k j i) -> p k j i', k=cnt, j=win)
                                src_ir = ir_t[:, g['base'] + k0 * gl:
                                              g['base'] + (k0 + cnt) * gl]
                                irv = src_ir.rearrange(
                                    'p (k i) -> p k i', i=gl).unsqueeze(
                                    2).to_broadcast([128, cnt, win, gl])
                                jiv = ji_t[:, :win].unsqueeze(1).unsqueeze(
                                    3).to_broadcast([128, cnt, win, gl])
                                nc.vector.tensor_tensor(
                                    out=ov, in0=jiv, in1=irv,
                                    op=mybir.AluOpType.is_equal)
                            ohs.append(oh)
                        # chunk-major so consecutive matmuls share lhsT
                        # (real HW reloads stationary weights per matmul)
                        for kk in range(cnt):
                            k = k0 + kk
                            for gi, g in enumerate(pss):
                                gl = len(g['angles'])
                                win = g['win']
                                lo = int(g['lo'][k])
                                nc.tensor.matmul(
                                    out=ptiles[gi][:,
                                                   gl * lo:gl * (lo + win)],
                                    lhsT=h1T[:, k * 128:(k + 1) * 128],
                                    rhs=ohs[gi][:, kk * win * gl:
                                                (kk + 1) * win * gl],
                                    start=False, stop=False,
                                    skip_group_check=True)
                    # copy accumulators into conv2 input (de-interleave)
                    for gi, g in enumerate(pss):
                        gl = len(g['angles'])
                        a0 = g['a0']
                        pv = ptiles[gi][:, :gl * 128].rearrange(
                            'p (r s) -> p s r', s=gl)
                        dv = dht_pad[:].rearrange('c (a r) -> c a r', a=PADW)
                        nc.scalar.activation(
                            out=dv[:, a0 + 1:a0 + 1 + gl, 1:1 + R],
                            in_=pv[:, :, :R], func=COPY)

                rm_passes = [p for p in passes if p[0]['layout'] == 'rm']
                cm_passes = [p for p in passes if p[0]['layout'] == 'cm']
                for pss in rm_passes:
                    emit_pass(pss)
                # cm prep runs under the rm passes: the strided h1->h1cm copy
                # on ACT, then the cm transposes follow the rm matmul stream
                nc.scalar.activation(
                    out=h1cm[:],
                    in_=h1[:].rearrange('c (y x) -> c x y', y=H, x=W),
                    func=COPY)
                transposes(h1cm, h1T_cm)
                if first_rep[0]:
                    first_rep[0] = False
                    for t9 in range(9):
                        nc.sync.dma_start(
                            out=w2_t[:, t9 * 128:(t9 + 1) * 128],
                            in_=w2_d.ap()[t9])
                        nc.sync.dma_start(
                            out=w3_t[:, t9 * 128:(t9 + 1) * 128],
                            in_=w3_d.ap()[t9])
                for pss in cm_passes:
                    emit_pass(pss)

                # ------------------------------------ conv2 / conv3
                # conv psum tiles share the DHT accumulator slots (same
                # tag), so conv2 chunks whose input rows are already
                # written (the rm-angle band) can fill PE gaps during the
                # cm passes.
                AR = 4  # angle rows per psum chunk
                for conv_i, (w_t, b_t, src_t) in enumerate(
                        ((w2_t, b2_t, dht_pad), (w3_t, b3_t, h2_pad))):
                    sv = src_t[:].rearrange('c (a r) -> c a r', a=PADW)
                    if conv_i == 0:
                        # rm-band chunks first: their input rows are done
                        # before the cm passes run, so they can fill PE gaps
                        corder = list(range(7, 18)) + \
                            [c for c in range(A // AR) if not 7 <= c < 18]
                    else:
                        corder = list(range(A // AR))
                    for c in corder:
                        a0 = c * AR
                        ps = psd.tile([128, 512], f32, tag='psd',
                                      space='PSUM')
                        for t9 in range(9):
                            dy, dx = divmod(t9, 3)
                            nc.tensor.matmul(
                                out=ps[:, :AR * R],
                                lhsT=w_t[:, t9 * 128:(t9 + 1) * 128],
                                rhs=sv[:, a0 + dy:a0 + dy + AR, dx:dx + R],
                                start=(t9 == 0), stop=(t9 == 8))
                        pv = ps[:, :AR * R].rearrange('p (a r) -> p a r',
                                                      a=AR)
                        if conv_i == 0:
                            hv = h2_pad[:].rearrange('c (a r) -> c a r',
                                                     a=PADW)
                            nc.scalar.activation(
                                out=hv[:, a0 + 1:a0 + 1 + AR, 1:1 + R],
                                in_=pv[:], func=RELU, bias=b_t[:, :1],
                                scale=1.0)
                            if debug_outs and c == A // AR - 1:
                                nc.sync.dma_start(out=dbg_h2_d.ap(),
                                                  in_=h2_pad[:])
                        else:
                            ob = outb_pool.tile([128, AR * R], f32,
                                                tag='outb')
                            ov = ob[:].rearrange('p (a r) -> p a r', a=AR)
                            nc.scalar.activation(out=ov[:], in_=pv[:],
                                                 func=RELU, bias=b_t[:, :1],
                                                 scale=1.0)
                            nc.sync.dma_start(
                                out=out_d.ap()[:, a0 * R:(a0 + AR) * R],
                                in_=ob[:])

          if debug_outs:
            nc.sync.dma_start(out=dbg_h1t_d.ap(), in_=h1T_rm[:])
            nc.sync.dma_start(out=dbg_dht_d.ap(), in_=dht_pad[:])

        for _rep in range(repeat):
            emit_pipeline()

    if split_waits:
        _split_multi_waits(nc, mybir)
    _PROGRAM_CACHE[key] = nc
    return nc


# ----------------------------------------------------------------------------
# entry point
# ----------------------------------------------------------------------------

def make_in_maps(inputs):
    T = _dht_tables()
    x = np.asarray(inputs['x'], np.float32)
    w1p, bias1, w2p, bias2, w3p, bias3, ident = _prep_weights(
        *[np.asarray(inputs[k], np.float32) for k in
          ('w1', 'b1', 'g1', 'be1', 'm1', 'v1',
           'w2', 'b2', 'g2', 'be2', 'm2', 'v2',
           'w3', 'b3', 'g3', 'be3', 'm3', 'v3')])
    common = dict(w1p=w1p, bias1=bias1, w2p=w2p, bias2=bias2, w3p=w3p,
                  bias3=bias3, ident=ident, idxrel=T['idxrel'],
                  jiota=T['jiota'], ohtable=T['ohtable'])
    return [
        {'x': np.ascontiguousarray(x[n]).reshape(CIN, HW).astype(BF16),
         **common}
        for n in range(N)
    ]


def run(inputs, trace=False):
    from concourse.bass_utils import run_bass_kernel_spmd

    nc = _build_program()
    in_maps = make_in_maps(inputs)
    res = run_bass_kernel_spmd(nc, in_maps, core_ids=list(range(N)),
                               trace=trace)
    out = np.stack([res.results[n]['out'].reshape(CMID, H, W)
                    for n in range(N)], axis=0)
    return out.astype(np.float32), res


def kernel(**inputs):
    out, _ = run(inputs, trace=False)
    return out
